# revision 6
# baseline (speedup 1.0000x reference)
"""GAT-style attention head (nn_AttentionHead) on 8 Trainium2 NeuronCores.

Math (reference):
    h  = x @ W.T                      [N, 128]
    s1 = h @ A1.T ; s2 = h @ A2.T     [N, 1]
    e[i,j]   = where(adj[i,j]>0, s1[i]+s2[j], -9e15)
    attn     = softmax(leaky_relu(e, 0.2), axis=1)
    out      = attn @ h

Device strategy (row-sharded across 8 cores, 1280 dest rows each):
  * transposed score layout [partition = j (source node), free = i (local dest)]
  * leaky_relu(s) = 0.2*s + 0.8*relu(s); inside a softmax row (fixed i) any
    per-i factor cancels, so exp(0.2*s1_i) is dropped:
        pm[j,i] = mask[j,i] * exp(0.2*s2_j + relu(0.8*(s1_i + s2_j)))
    Masked entries of the reference softmax are exactly 0 in fp32 (exp
    underflow), so multiplying by the 0/1 mask is exact.
  * relu stage: one fused DVE tensor_scalar (add + max) on f32
  * exp stage: one ScalarE activation with per-partition bias -> bf16
  * mask stage: one DVE tensor_tensor mult (bf16 x bf16 -> bf16, 2x mode)
  * softmax denominator: ones.T @ pm on the TensorEngine (PSUM accumulate)
  * numerator: h_chunk.T @ pm accumulated over all 80 j-chunks in PSUM
  * h-compute (bf16, fused [h | 0.8*s2 | 0.2*s2] rhs) is interleaved with the
    attention loop so PE/ACT/DVE/DMA all overlap.
"""

import os
from contextlib import ExitStack

import numpy as np
import ml_dtypes

import concourse.bass as bass
import concourse.bacc as bacc
import concourse.tile as tile
import concourse.mybir as mybir
from concourse.alu_op_type import AluOpType
from concourse.bass_utils import run_bass_kernel_spmd

# Problem constants (hardcoded per contract)
N = 10000
IN_F = 512
OUT_F = 128
NCORES = 8

NP = 10240          # padded node count (j dimension), 80 chunks of 128
IL = 1280           # local destination rows per core (8 * 1280 = NP)
JCH = NP // 128     # 80 j-chunks
KCH = IN_F // 128   # 4 contraction chunks for h = x @ W.T
SUBS = [(0, 512), (512, 1024), (1024, 1280)]  # psum free-dim sub-tiles
GB = 4              # j-chunks per batched DMA (mask / x)
LAG = 4             # h-compute chunks ahead of the attention loop

F32 = mybir.dt.float32
BF16 = mybir.dt.bfloat16

LAST_EXEC_NS = None
LAST_RESULTS = None

_prog = None


def _build_program():
    nc = bacc.Bacc("TRN2")

    d_xTr = nc.dram_tensor("xTr", [128, KCH, NP], BF16, kind="ExternalInput")
    d_wc = nc.dram_tensor("wcomb", [128, KCH, 130], BF16, kind="ExternalInput")
    d_wnat = nc.dram_tensor("wnat", [OUT_F, IN_F], F32, kind="ExternalInput")
    d_a12 = nc.dram_tensor("a12T", [OUT_F, 2], F32, kind="ExternalInput")
    d_xl = nc.dram_tensor("xlTr", [128, KCH, IL], F32, kind="ExternalInput")
    d_mb = nc.dram_tensor("maskb", [NP, IL], BF16, kind="ExternalInput")
    d_br08 = nc.dram_tensor("br08", [1, 128], F32, kind="ExternalInput")
    d_ones_bf = nc.dram_tensor("ones_bf", [128, 1], BF16, kind="ExternalInput")
    d_ones1 = nc.dram_tensor("ones1", [1, 128], F32, kind="ExternalInput")
    d_outT = nc.dram_tensor("outT", [OUT_F, IL], F32, kind="ExternalOutput")

    with tile.TileContext(nc) as tc, ExitStack() as ctx:
        consts = ctx.enter_context(tc.tile_pool(name="consts", bufs=1))
        xpool = ctx.enter_context(tc.tile_pool(name="xpool", bufs=3))
        mpool = ctx.enter_context(tc.tile_pool(name="mpool", bufs=3))
        rpool = ctx.enter_context(tc.tile_pool(name="rpool", bufs=3))
        epool = ctx.enter_context(tc.tile_pool(name="epool", bufs=3))
        pmpool = ctx.enter_context(tc.tile_pool(name="pmpool", bufs=3))
        fin = ctx.enter_context(tc.tile_pool(name="fin", bufs=2))
        psum = ctx.enter_context(tc.tile_pool(name="psum", bufs=2, space="PSUM"))

        # ---- constants into SBUF
        wc = consts.tile([128, KCH, 130], BF16)   # [W.T | 0.8 w2 | 0.2 w2] chunks
        wnat = consts.tile([OUT_F, IN_F], F32)
        a12 = consts.tile([OUT_F, 2], F32)
        xl = consts.tile([128, KCH, IL], F32)
        br08 = consts.tile([1, 128], F32)
        ones_bf = consts.tile([128, 1], BF16)
        ones1 = consts.tile([1, 128], F32)
        nc.sync.dma_start(wc[:, :, 0:128], d_wc[:, :, 0:128])
        nc.sync.dma_start(wnat[:], d_wnat[:])
        nc.sync.dma_start(a12[:], d_a12[:])
        nc.sync.dma_start(xl[:], d_xl[:])
        nc.sync.dma_start(br08[:], d_br08[:])
        nc.sync.dma_start(ones_bf[:], d_ones_bf[:])
        nc.sync.dma_start(ones1[:], d_ones1[:])

        h_sb = consts.tile([128, NP], BF16)
        s2cc = consts.tile([128, JCH, 2], F32)    # per chunk: [0.8*s2 | 0.2*s2]
        w1 = consts.tile([128, KCH, 1], F32)      # w1.T chunks (f32, for s1)
        s1row = consts.tile([1, IL], F32)
        S1b08 = consts.tile([128, IL], F32)
        den_sb = consts.tile([1, IL], F32)

        # ---- stage A: w1 (f32) and scaled-w2 columns of wc (bf16)
        for k in range(KCH):
            w12ps = psum.tile([128, 2], F32, tag="hps", name="w12ps")
            nc.tensor.matmul(w12ps[:], wnat[:, k * 128:(k + 1) * 128], a12[:],
                             start=True, stop=True)
            nc.vector.tensor_copy(w1[:, k, :], w12ps[:, 0:1])
            nc.vector.tensor_scalar_mul(wc[:, k, 128:129], w12ps[:, 1:2], 0.8)
            nc.vector.tensor_scalar_mul(wc[:, k, 129:130], w12ps[:, 1:2], 0.2)

        # ---- stage C: s1 row for local dest rows, broadcast 0.8*s1
        for (lo, hi) in SUBS:
            s1ps = psum.tile([1, 512], F32, tag="hps", name="s1ps")
            for k in range(KCH):
                nc.tensor.matmul(s1ps[:, 0:hi - lo], w1[:, k, :], xl[:, k, lo:hi],
                                 start=(k == 0), stop=(k == KCH - 1))
            nc.vector.tensor_copy(s1row[:, lo:hi], s1ps[:, 0:hi - lo])
        for (lo, hi) in SUBS:
            bps = psum.tile([128, 512], F32, tag="hps", name="bps")
            nc.tensor.matmul(bps[:, 0:hi - lo], br08[:], s1row[:, lo:hi],
                             start=True, stop=True)
            nc.vector.tensor_copy(S1b08[:, lo:hi], bps[:, 0:hi - lo])

        # ---- interleaved: h-compute chunk `step` + attention chunk `step-LAG`
        den_ps = [psum.tile([1, hi - lo], F32, tag=f"den{i}", name=f"den{i}",
                            bufs=1)
                  for i, (lo, hi) in enumerate(SUBS)]
        out_ps = [psum.tile([128, hi - lo], F32, tag=f"out{i}", name=f"out{i}",
                            bufs=1)
                  for i, (lo, hi) in enumerate(SUBS)]

        xt4 = None
        mb_tiles = {}
        for step in range(JCH + LAG):
            if step < JCH and step % GB == 0:
                b = step // GB
                mb = mpool.tile([128, GB, IL], BF16, name="mb4")
                nc.sync.dma_start(
                    mb[:],
                    d_mb[b * GB * 128:(b + 1) * GB * 128, :].rearrange(
                        "(g p) i -> p g i", p=128))
                mb_tiles[b] = mb
            if step < JCH:
                c = step
                if c % GB == 0:
                    xt4 = xpool.tile([128, KCH, GB * 128], BF16, name="xt4")
                    nc.sync.dma_start(
                        xt4[:], d_xTr[:, :, c * 128:(c + GB) * 128])
                co = (c % GB) * 128
                hps = psum.tile([128, 130], F32, tag="hps", name="hps")
                for k in range(KCH):
                    nc.tensor.matmul(hps[:], xt4[:, k, co:co + 128], wc[:, k, :],
                                     start=(k == 0), stop=(k == KCH - 1))
                nc.scalar.copy(h_sb[:, c * 128:(c + 1) * 128], hps[:, 0:128])
                nc.vector.tensor_copy(s2cc[:, c, :], hps[:, 128:130])

            if step >= LAG:
                jc = step - LAG
                mb4 = mb_tiles.pop(jc // GB) if jc % GB == GB - 1 else mb_tiles[jc // GB]
                g = jc % GB
                r = rpool.tile([128, IL], F32, name="r")
                relu_eng = nc.gpsimd if jc % 8 < 3 else nc.vector
                relu_eng.tensor_scalar(r[:], S1b08[:], s2cc[:, jc, 0:1], 0.0,
                                       AluOpType.add, AluOpType.max)
                e = epool.tile([128, IL], BF16, name="e")
                nc.scalar.activation(e[:], r[:],
                                     mybir.ActivationFunctionType.Exp,
                                     bias=s2cc[:, jc, 1:2], scale=1.0)
                pm = pmpool.tile([128, IL], BF16, name="pm")
                nc.vector.tensor_tensor(pm[:], e[:], mb4[:, g, :], AluOpType.mult)

                hj = h_sb[:, jc * 128:(jc + 1) * 128]
                for i, (lo, hi) in enumerate(SUBS):
                    nc.tensor.matmul(den_ps[i][:], ones_bf[:], pm[:, lo:hi],
                                     start=(jc == 0), stop=(jc == JCH - 1))
                for i, (lo, hi) in enumerate(SUBS):
                    nc.tensor.matmul(out_ps[i][:], hj, pm[:, lo:hi],
                                     start=(jc == 0), stop=(jc == JCH - 1))

        # ---- finale: normalize and write out (transposed [f, i])
        for i, (lo, hi) in enumerate(SUBS):
            nc.vector.tensor_copy(den_sb[:, lo:hi], den_ps[i][:])
        nc.vector.tensor_scalar_add(den_sb[:], den_sb[:], 1e-30)
        for i, (lo, hi) in enumerate(SUBS):
            rbps = psum.tile([128, 512], F32, tag="hps", name="rbps")
            nc.tensor.matmul(rbps[:, 0:hi - lo], ones1[:], den_sb[:, lo:hi],
                             start=True, stop=True)
            rb_sb = fin.tile([128, 512], F32, tag="rbsb", name="rb_sb")
            nc.vector.reciprocal_approx_fast(rb_sb[:, 0:hi - lo],
                                             rbps[:, 0:hi - lo])
            osb = fin.tile([128, 512], F32, tag="osb", name="osb")
            nc.vector.tensor_tensor(osb[:, 0:hi - lo], out_ps[i][:],
                                    rb_sb[:, 0:hi - lo], AluOpType.mult)
            nc.sync.dma_start(d_outT[:, lo:hi], osb[:, 0:hi - lo])

    nc.finalize()
    return nc


def get_program():
    global _prog
    if _prog is None:
        _prog = _build_program()
    return _prog


def prep_host_inputs(x, edge_index, W, A1, A2):
    """Build the per-core in_maps (host-side sharding + layout prep)."""
    x = np.asarray(x, np.float32)
    W = np.asarray(W, np.float32)
    A1 = np.asarray(A1, np.float32)
    A2 = np.asarray(A2, np.float32)
    ei = np.asarray(edge_index)

    x_pad = np.zeros((NP, IN_F), np.float32)
    x_pad[:N] = x
    # xTr[p, k, n] = x_pad[n, 128k + p]
    xTr = np.ascontiguousarray(x_pad.T.reshape(KCH, 128, NP).transpose(1, 0, 2))
    xTr_bf = xTr.astype(ml_dtypes.bfloat16)
    # wcomb[p, k, 0:128] = W[f, 128k + p]; cols 128/129 filled on device
    wcomb = np.zeros((128, KCH, 130), ml_dtypes.bfloat16)
    wcomb[:, :, 0:128] = W.T.reshape(KCH, 128, OUT_F).transpose(1, 0, 2)
    a12T = np.ascontiguousarray(np.stack([A1[0], A2[0]], axis=1))  # [128, 2]

    # transposed adjacency mask: maskb[j, i] = 1 iff edge (dest=i, src=j)
    M8 = np.zeros((NP, NP), ml_dtypes.bfloat16)
    M8[ei[1], ei[0]] = 1

    br08 = np.full((1, 128), 0.8, np.float32)
    ones_bf = np.ones((128, 1), ml_dtypes.bfloat16)
    ones1 = np.ones((1, 128), np.float32)

    in_maps = []
    for c in range(NCORES):
        lo = c * IL
        in_maps.append({
            "xTr": xTr_bf,
            "wcomb": wcomb,
            "wnat": W,
            "a12T": a12T,
            "xlTr": np.ascontiguousarray(xTr[:, :, lo:lo + IL]),
            "maskb": np.ascontiguousarray(M8[:, lo:lo + IL]),
            "br08": br08,
            "ones_bf": ones_bf,
            "ones1": ones1,
        })
    return in_maps


def kernel(x, edge_index, W, A1, A2):
    global LAST_EXEC_NS, LAST_RESULTS
    in_maps = prep_host_inputs(x, edge_index, W, A1, A2)
    nc = get_program()

    trace = os.environ.get("KERNEL_TRACE", "0") == "1"
    res = run_bass_kernel_spmd(
        nc, in_maps, core_ids=list(range(NCORES)), trace=trace,
    )
    LAST_RESULTS = res
    LAST_EXEC_NS = res.exec_time_ns

    out = np.empty((NP, OUT_F), np.float32)
    for c in range(NCORES):
        outT = res.results[c]["outT"]  # [OUT_F, IL]
        out[c * IL:(c + 1) * IL] = outT.T
    out = out[:N]

    # Reference semantics for isolated rows (no out-edges): uniform attention.
    ei = np.asarray(edge_index)
    deg = np.bincount(np.asarray(ei[0], np.int64), minlength=N)
    if (deg == 0).any():
        h_host = np.asarray(x, np.float32) @ np.asarray(W, np.float32).T
        out[deg == 0] = h_host.mean(axis=0)
    return out


# revision 7
# speedup vs baseline: 3.8532x; 3.8532x over previous
"""GAT-style attention head (nn_AttentionHead) on 8 Trainium2 NeuronCores.

Math (reference):
    h  = x @ W.T                      [N, 128]
    s1 = h @ A1.T ; s2 = h @ A2.T     [N, 1]
    e[i,j]   = where(adj[i,j]>0, s1[i]+s2[j], -9e15)
    attn     = softmax(leaky_relu(e, 0.2), axis=1)
    out      = attn @ h

Device strategy (row-sharded across 8 cores, 1280 dest rows each):
  * transposed score layout [partition = j (source node), free = i (local dest)]
  * leaky_relu(s) = 0.2*s + 0.8*relu(s); inside a softmax row (fixed i) any
    per-i factor cancels, so exp(0.2*s1_i) is dropped:
        pm[j,i] = mask[j,i] * exp(0.2*s2_j + relu(0.8*(s1_i + s2_j)))
    Masked entries of the reference softmax are exactly 0 in fp32 (exp
    underflow), so multiplying by the 0/1 mask is exact.
  * relu stage: one fused DVE tensor_scalar (add + max) on f32
  * exp stage: one ScalarE activation with per-partition bias -> bf16
  * mask stage: one DVE tensor_tensor mult (bf16 x bf16 -> bf16, 2x mode)
  * softmax denominator: ones.T @ pm on the TensorEngine (PSUM accumulate)
  * numerator: h_chunk.T @ pm accumulated over all 80 j-chunks in PSUM
  * h-compute (bf16, fused [h | 0.8*s2 | 0.2*s2] rhs) is interleaved with the
    attention loop so PE/ACT/DVE/DMA all overlap.
"""

import os
from contextlib import ExitStack

import numpy as np
import ml_dtypes

import concourse.bass as bass
import concourse.bacc as bacc
import concourse.tile as tile
import concourse.mybir as mybir
from concourse.alu_op_type import AluOpType
from concourse.bass_utils import run_bass_kernel_spmd

# Problem constants (hardcoded per contract)
N = 10000
IN_F = 512
OUT_F = 128
NCORES = 8

NP = 10240          # padded node count (j dimension), 80 chunks of 128
IL = 1280           # local destination rows per core (8 * 1280 = NP)
JCH = NP // 128     # 80 j-chunks
KCH = IN_F // 128   # 4 contraction chunks for h = x @ W.T
SUBS = [(0, 512), (512, 1024), (1024, 1280)]  # psum free-dim sub-tiles
GB = 4              # j-chunks per batched DMA (mask / x)
LAG = 4             # h-compute chunks ahead of the attention loop

F32 = mybir.dt.float32
BF16 = mybir.dt.bfloat16

LAST_EXEC_NS = None
LAST_RESULTS = None

_prog = None


def _build_program():
    nc = bacc.Bacc("TRN2")

    d_xTr = nc.dram_tensor("xTr", [128, KCH, NP], BF16, kind="ExternalInput")
    d_wc = nc.dram_tensor("wcomb", [128, KCH, 130], BF16, kind="ExternalInput")
    d_wnat = nc.dram_tensor("wnat", [OUT_F, IN_F], F32, kind="ExternalInput")
    d_a12 = nc.dram_tensor("a12T", [OUT_F, 2], F32, kind="ExternalInput")
    d_xl = nc.dram_tensor("xlTr", [128, KCH, IL], F32, kind="ExternalInput")
    d_mb = nc.dram_tensor("maskb", [NP, IL], BF16, kind="ExternalInput")
    d_br08 = nc.dram_tensor("br08", [1, 128], F32, kind="ExternalInput")
    d_ones_bf = nc.dram_tensor("ones_bf", [128, 1], BF16, kind="ExternalInput")
    d_ones1 = nc.dram_tensor("ones1", [1, 128], F32, kind="ExternalInput")
    d_outT = nc.dram_tensor("outT", [OUT_F, IL], F32, kind="ExternalOutput")

    with tile.TileContext(nc) as tc, ExitStack() as ctx:
        consts = ctx.enter_context(tc.tile_pool(name="consts", bufs=1))
        xpool = ctx.enter_context(tc.tile_pool(name="xpool", bufs=3))
        mpool = ctx.enter_context(tc.tile_pool(name="mpool", bufs=3))
        rpool = ctx.enter_context(tc.tile_pool(name="rpool", bufs=3))
        epool = ctx.enter_context(tc.tile_pool(name="epool", bufs=3))
        pmpool = ctx.enter_context(tc.tile_pool(name="pmpool", bufs=3))
        fin = ctx.enter_context(tc.tile_pool(name="fin", bufs=2))
        psum = ctx.enter_context(tc.tile_pool(name="psum", bufs=2, space="PSUM"))

        # ---- constants into SBUF
        wc = consts.tile([128, KCH, 130], BF16)   # [W.T | 0.8 w2 | 0.2 w2] chunks
        wnat = consts.tile([OUT_F, IN_F], F32)
        a12 = consts.tile([OUT_F, 2], F32)
        xl = consts.tile([128, KCH, IL], F32)
        br08 = consts.tile([1, 128], F32)
        ones_bf = consts.tile([128, 1], BF16)
        ones1 = consts.tile([1, 128], F32)
        nc.sync.dma_start(wc[:, :, 0:128], d_wc[:, :, 0:128])
        nc.sync.dma_start(wnat[:], d_wnat[:])
        nc.sync.dma_start(a12[:], d_a12[:])
        nc.sync.dma_start(xl[:], d_xl[:])
        nc.sync.dma_start(br08[:], d_br08[:])
        nc.sync.dma_start(ones_bf[:], d_ones_bf[:])
        nc.sync.dma_start(ones1[:], d_ones1[:])

        h_sb = consts.tile([128, NP], BF16)
        s2cc = consts.tile([128, JCH, 2], F32)    # per chunk: [0.8*s2 | 0.2*s2]
        w1 = consts.tile([128, KCH, 1], F32)      # w1.T chunks (f32, for s1)
        s1row = consts.tile([1, IL], F32)
        S1b08 = consts.tile([128, IL], F32)
        den_sb = consts.tile([1, IL], F32)

        # ---- stage A: w1 (f32) and scaled-w2 columns of wc (bf16)
        for k in range(KCH):
            w12ps = psum.tile([128, 2], F32, tag="hps", name="w12ps")
            nc.tensor.matmul(w12ps[:], wnat[:, k * 128:(k + 1) * 128], a12[:],
                             start=True, stop=True)
            nc.vector.tensor_copy(w1[:, k, :], w12ps[:, 0:1])
            nc.vector.tensor_scalar_mul(wc[:, k, 128:129], w12ps[:, 1:2], 0.8)
            nc.vector.tensor_scalar_mul(wc[:, k, 129:130], w12ps[:, 1:2], 0.2)

        # ---- stage C: s1 row for local dest rows, broadcast 0.8*s1
        for (lo, hi) in SUBS:
            s1ps = psum.tile([1, 512], F32, tag="hps", name="s1ps")
            for k in range(KCH):
                nc.tensor.matmul(s1ps[:, 0:hi - lo], w1[:, k, :], xl[:, k, lo:hi],
                                 start=(k == 0), stop=(k == KCH - 1))
            nc.vector.tensor_copy(s1row[:, lo:hi], s1ps[:, 0:hi - lo])
        for (lo, hi) in SUBS:
            bps = psum.tile([128, 512], F32, tag="hps", name="bps")
            nc.tensor.matmul(bps[:, 0:hi - lo], br08[:], s1row[:, lo:hi],
                             start=True, stop=True)
            nc.vector.tensor_copy(S1b08[:, lo:hi], bps[:, 0:hi - lo])

        # ---- interleaved: h-compute chunk `step` + attention chunk `step-LAG`
        den_ps = [psum.tile([1, hi - lo], F32, tag=f"den{i}", name=f"den{i}",
                            bufs=1)
                  for i, (lo, hi) in enumerate(SUBS)]
        out_ps = [psum.tile([128, hi - lo], F32, tag=f"out{i}", name=f"out{i}",
                            bufs=1)
                  for i, (lo, hi) in enumerate(SUBS)]

        xt4 = None
        mb_tiles = {}
        for step in range(JCH + LAG):
            if step < JCH and step % GB == 0:
                b = step // GB
                mb = mpool.tile([128, GB, IL], BF16, name="mb4")
                nc.sync.dma_start(
                    mb[:],
                    d_mb[b * GB * 128:(b + 1) * GB * 128, :].rearrange(
                        "(g p) i -> p g i", p=128))
                mb_tiles[b] = mb
            if step < JCH:
                c = step
                if c % GB == 0:
                    xt4 = xpool.tile([128, KCH, GB * 128], BF16, name="xt4")
                    nc.sync.dma_start(
                        xt4[:], d_xTr[:, :, c * 128:(c + GB) * 128])
                co = (c % GB) * 128
                hps = psum.tile([128, 130], F32, tag="hps", name="hps")
                for k in range(KCH):
                    nc.tensor.matmul(hps[:], xt4[:, k, co:co + 128], wc[:, k, :],
                                     start=(k == 0), stop=(k == KCH - 1))
                nc.scalar.copy(h_sb[:, c * 128:(c + 1) * 128], hps[:, 0:128])
                nc.vector.tensor_copy(s2cc[:, c, :], hps[:, 128:130])

            if step >= LAG:
                jc = step - LAG
                mb4 = mb_tiles.pop(jc // GB) if jc % GB == GB - 1 else mb_tiles[jc // GB]
                g = jc % GB
                r = rpool.tile([128, IL], F32, name="r")
                nc.vector.tensor_scalar(r[:], S1b08[:], s2cc[:, jc, 0:1], 0.0,
                                        AluOpType.add, AluOpType.max)
                e = epool.tile([128, IL], BF16, name="e")
                nc.scalar.activation(e[:], r[:],
                                     mybir.ActivationFunctionType.Exp,
                                     bias=s2cc[:, jc, 1:2], scale=1.0)
                pm = pmpool.tile([128, IL], BF16, name="pm")
                nc.vector.tensor_tensor(pm[:], e[:], mb4[:, g, :], AluOpType.mult)

                hj = h_sb[:, jc * 128:(jc + 1) * 128]
                for i, (lo, hi) in enumerate(SUBS):
                    nc.tensor.matmul(den_ps[i][:], ones_bf[:], pm[:, lo:hi],
                                     start=(jc == 0), stop=(jc == JCH - 1))
                for i, (lo, hi) in enumerate(SUBS):
                    nc.tensor.matmul(out_ps[i][:], hj, pm[:, lo:hi],
                                     start=(jc == 0), stop=(jc == JCH - 1))

        # ---- finale: normalize and write out (transposed [f, i])
        for i, (lo, hi) in enumerate(SUBS):
            nc.vector.tensor_copy(den_sb[:, lo:hi], den_ps[i][:])
        nc.vector.tensor_scalar_add(den_sb[:], den_sb[:], 1e-30)
        for i, (lo, hi) in enumerate(SUBS):
            rbps = psum.tile([128, 512], F32, tag="hps", name="rbps")
            nc.tensor.matmul(rbps[:, 0:hi - lo], ones1[:], den_sb[:, lo:hi],
                             start=True, stop=True)
            rb_sb = fin.tile([128, 512], F32, tag="rbsb", name="rb_sb")
            nc.vector.reciprocal_approx_fast(rb_sb[:, 0:hi - lo],
                                             rbps[:, 0:hi - lo])
            osb = fin.tile([128, 512], F32, tag="osb", name="osb")
            nc.vector.tensor_tensor(osb[:, 0:hi - lo], out_ps[i][:],
                                    rb_sb[:, 0:hi - lo], AluOpType.mult)
            nc.sync.dma_start(d_outT[:, lo:hi], osb[:, 0:hi - lo])

    nc.finalize()
    return nc


def get_program():
    global _prog
    if _prog is None:
        _prog = _build_program()
    return _prog


def prep_host_inputs(x, edge_index, W, A1, A2):
    """Build the per-core in_maps (host-side sharding + layout prep)."""
    x = np.asarray(x, np.float32)
    W = np.asarray(W, np.float32)
    A1 = np.asarray(A1, np.float32)
    A2 = np.asarray(A2, np.float32)
    ei = np.asarray(edge_index)

    x_pad = np.zeros((NP, IN_F), np.float32)
    x_pad[:N] = x
    # xTr[p, k, n] = x_pad[n, 128k + p]
    xTr = np.ascontiguousarray(x_pad.T.reshape(KCH, 128, NP).transpose(1, 0, 2))
    xTr_bf = xTr.astype(ml_dtypes.bfloat16)
    # wcomb[p, k, 0:128] = W[f, 128k + p]; cols 128/129 filled on device
    wcomb = np.zeros((128, KCH, 130), ml_dtypes.bfloat16)
    wcomb[:, :, 0:128] = W.T.reshape(KCH, 128, OUT_F).transpose(1, 0, 2)
    a12T = np.ascontiguousarray(np.stack([A1[0], A2[0]], axis=1))  # [128, 2]

    # transposed adjacency mask: maskb[j, i] = 1 iff edge (dest=i, src=j)
    M8 = np.zeros((NP, NP), ml_dtypes.bfloat16)
    M8[ei[1], ei[0]] = 1

    br08 = np.full((1, 128), 0.8, np.float32)
    ones_bf = np.ones((128, 1), ml_dtypes.bfloat16)
    ones1 = np.ones((1, 128), np.float32)

    in_maps = []
    for c in range(NCORES):
        lo = c * IL
        in_maps.append({
            "xTr": xTr_bf,
            "wcomb": wcomb,
            "wnat": W,
            "a12T": a12T,
            "xlTr": np.ascontiguousarray(xTr[:, :, lo:lo + IL]),
            "maskb": np.ascontiguousarray(M8[:, lo:lo + IL]),
            "br08": br08,
            "ones_bf": ones_bf,
            "ones1": ones1,
        })
    return in_maps


def kernel(x, edge_index, W, A1, A2):
    global LAST_EXEC_NS, LAST_RESULTS
    in_maps = prep_host_inputs(x, edge_index, W, A1, A2)
    nc = get_program()

    trace = os.environ.get("KERNEL_TRACE", "0") == "1"
    res = run_bass_kernel_spmd(
        nc, in_maps, core_ids=list(range(NCORES)), trace=trace,
    )
    LAST_RESULTS = res
    LAST_EXEC_NS = res.exec_time_ns

    out = np.empty((NP, OUT_F), np.float32)
    for c in range(NCORES):
        outT = res.results[c]["outT"]  # [OUT_F, IL]
        out[c * IL:(c + 1) * IL] = outT.T
    out = out[:N]

    # Reference semantics for isolated rows (no out-edges): uniform attention.
    ei = np.asarray(edge_index)
    deg = np.bincount(np.asarray(ei[0], np.int64), minlength=N)
    if (deg == 0).any():
        h_host = np.asarray(x, np.float32) @ np.asarray(W, np.float32).T
        out[deg == 0] = h_host.mean(axis=0)
    return out
